# revision 18
# baseline (speedup 1.0000x reference)
"""Multi-head causal attention (B=2, S=2048, D=1024, H=16) on 8 TRN2 cores.

Sharding: batch x head-group. Core c handles batch c//4 and heads
4*(c%4) .. 4*(c%4)+3. Each core computes its 4 heads' attention plus the
partial output projection; the host sums the 4 partials per batch and adds
the folded bias vector.
"""

import ml_dtypes
import numpy as np
from contextlib import ExitStack

NP_BF16 = np.float16

import concourse.bass as bass
import concourse.tile as tile
from concourse import mybir, bacc
from concourse.bass_utils import run_bass_kernel_spmd

B, S, D, H = 2, 2048, 1024, 16
DEPTH = D // H            # 64
HPC = 4                   # heads per core
DHC = HPC * DEPTH         # 256 head-dims per core
N_CORES = 8
P = 128
KT = D // P               # 8 contraction tiles for projections
ST = S // P               # 16 sequence tiles
F32 = mybir.dt.float32
BF16 = mybir.dt.bfloat16
MMDT = mybir.dt.float16   # dtype for matmul operands (PSUM accum stays fp32)
NEG = -1.0e9


def _build_program():
    nc = bacc.Bacc("TRN2", target_bir_lowering=False, debug=False)

    qT = nc.dram_tensor("qT", [D, S], MMDT, kind="ExternalInput").ap()
    kT = nc.dram_tensor("kT", [D, S], MMDT, kind="ExternalInput").ap()
    vT = nc.dram_tensor("vT", [D, S], MMDT, kind="ExternalInput").ap()
    wq = nc.dram_tensor("wq", [D, DHC], MMDT, kind="ExternalInput").ap()
    wk = nc.dram_tensor("wk", [D, DHC], MMDT, kind="ExternalInput").ap()
    wv = nc.dram_tensor("wv", [D, DHC], MMDT, kind="ExternalInput").ap()
    wo = nc.dram_tensor("wo", [DHC, D], MMDT, kind="ExternalInput").ap()
    qb = nc.dram_tensor("qb", [DHC], F32, kind="ExternalInput").ap()
    kb = nc.dram_tensor("kb", [DHC], F32, kind="ExternalInput").ap()
    mk = nc.dram_tensor("mk", [P, P], MMDT, kind="ExternalInput").ap()
    out = nc.dram_tensor("out", [S, D], F32, kind="ExternalOutput").ap()

    with tile.TileContext(nc) as tc, ExitStack() as ctx:
        persist = ctx.enter_context(tc.tile_pool(name="persist", bufs=1))

        # Persistent SBUF tensors
        qhT = [persist.tile([P, S], MMDT, tag=f"qhT{i}", name=f"qhT{i}") for i in range(2)]
        khT = [persist.tile([P, S], MMDT, tag=f"khT{i}", name=f"khT{i}") for i in range(2)]
        vh = [persist.tile([P, HPC, DEPTH + 1], MMDT, tag=f"vh{st}", name=f"vh{st}")
              for st in range(ST)]
        outT = [persist.tile([P, S], MMDT, tag=f"outT{i}", name=f"outT{i}") for i in range(2)]
        wq_sb = persist.tile([P, KT, DHC], MMDT, tag="wq")
        wk_sb = persist.tile([P, KT, DHC], MMDT, tag="wk")
        wv_sb = persist.tile([P, KT, DHC], MMDT, tag="wv")
        wo_sb = persist.tile([P, 2, D], MMDT, tag="wo")
        qb_sb = persist.tile([P, 2], F32, tag="qb")
        kb_sb = persist.tile([P, 2], F32, tag="kb")
        mk_sb = persist.tile([P, P], MMDT, tag="mk")
        ones_sb = persist.tile([1, 64], F32, tag="ones")
        nc.vector.memset(ones_sb[:], 1.0)

        nc.sync.dma_start(wq_sb[:], wq.rearrange("(ko ki) n -> ki ko n", ki=P))
        nc.sync.dma_start(qb_sb[:], qb.rearrange("(ko ki) -> ki ko", ki=P))

        # ---------------- q/k/v projections ----------------
        pctx = ExitStack()
        inp = pctx.enter_context(tc.tile_pool(name="inp", bufs=KT))
        with pctx:
            pp2 = ExitStack()
            pp = pp2.enter_context(
                tc.tile_pool(name="projps", bufs=4, space="PSUM"))

            for src_ap, wsb, bias_sb, dst in (
                (qT, wq_sb, qb_sb, qhT),
                (kT, wk_sb, kb_sb, khT),
            ):
                if src_ap is kT:
                    nc.sync.dma_start(
                        wk_sb[:], wk.rearrange("(ko ki) n -> ki ko n", ki=P))
                    nc.sync.dma_start(
                        kb_sb[:], kb.rearrange("(ko ki) -> ki ko", ki=P))
                    nc.sync.dma_start(
                        wv_sb[:], wv.rearrange("(ko ki) n -> ki ko n", ki=P))
                    nc.sync.dma_start(
                        wo_sb[:], wo.rearrange("(ko ki) n -> ki ko n", ki=P))
                    nc.sync.dma_start(mk_sb[:], mk)
                tiles = []
                for kt in range(KT):
                    t = inp.tile([P, S], MMDT, tag="inp", name="inp_t")
                    nc.sync.dma_start(t[:], src_ap[kt * P:(kt + 1) * P, :])
                    tiles.append(t)
                for dhb in range(2):
                    ps = [pp.tile([P, 512], F32, tag="projps", name="projps_t")
                          for _ in range(4)]
                    for kt in range(KT):
                        for sb in range(4):
                            nc.tensor.matmul(
                                ps[sb][:],
                                lhsT=wsb[:, kt, dhb * P:(dhb + 1) * P],
                                rhs=tiles[kt][:, sb * 512:(sb + 1) * 512],
                                start=(kt == 0), stop=(kt == KT - 1))
                    for sb in range(4):
                        nc.vector.tensor_scalar_add(
                            dst[dhb][:, sb * 512:(sb + 1) * 512],
                            ps[sb][:], bias_sb[:, dhb:dhb + 1])
            pp2.close()

            # ---------------- v projection ----------------
            with ExitStack() as vctx:
                vp = vctx.enter_context(
                    tc.tile_pool(name="vps", bufs=3, space="PSUM"))
                vtiles = []
                for kt in range(KT):
                    t = inp.tile([P, S], MMDT, tag="inp", name="inp_t")
                    nc.sync.dma_start(t[:], vT[kt * P:(kt + 1) * P, :])
                    vtiles.append(t)
                for st in range(ST):
                    pv = vp.tile([P, DHC], F32, tag="vps", name="vps_t")
                    for kt in range(KT):
                        nc.tensor.matmul(
                            pv[:],
                            lhsT=vtiles[kt][:, st * P:(st + 1) * P],
                            rhs=wv_sb[:, kt, :],
                            start=(kt == 0), stop=(kt == KT - 1))
                    nc.vector.tensor_copy(
                        vh[st][:, :, 0:DEPTH],
                        pv.rearrange("p (h c) -> p h c", h=HPC))
                    nc.vector.memset(vh[st][:, :, DEPTH:DEPTH + 1], 1.0)

        # ---------------- attention ----------------
        # Software-pipelined: scores+exp of head h+1 are emitted before the
        # A@V of head h, so the PE always has dense independent work and the
        # HAM clock gate stays at 8/8.
        with ExitStack() as actx:
            sc = actx.enter_context(
                tc.tile_pool(name="scps", bufs=3, space="PSUM"))
            ex = actx.enter_context(tc.tile_pool(name="exp", bufs=3))
            ns = actx.enter_context(tc.tile_pool(name="norm", bufs=2))

            ex_tiles = {}

            def phase_scores_pair(hb):
                heads = (2 * hb, 2 * hb + 1)
                for h in heads:
                    ex_tiles[h] = [ex.tile([P, S - jt * P], MMDT,
                                           tag=f"ex{jt}", name=f"ex{jt}_t")
                                   for jt in range(ST)]
                for jt in range(ST):
                    c0 = jt * P
                    for w0 in range(0, S, 1024):   # 1024-wide exp windows
                        w1 = w0 + 1024
                        lo = max(c0, w0)
                        if lo >= w1:
                            continue
                        pss = [sc.tile([P, 1024], F32, tag="scps",
                                       name="scps_t") for _ in heads]
                        # adjacent K=64 matmuls in row groups 0-1 / 2-3 run
                        # concurrently in the PE array
                        for p0 in range(w0, w1, 512):
                            a, b = max(lo, p0), p0 + 512
                            if a >= b:
                                continue
                            for h in heads:
                                rb = (h % 2) * 64
                                nc.tensor.matmul(
                                    pss[h % 2][:, a - w0:b - w0],
                                    lhsT=khT[hb][rb:rb + 64, c0:c0 + P],
                                    rhs=qhT[hb][rb:rb + 64, a:b],
                                    start=True, stop=True)
                        for h in heads:
                            exT = ex_tiles[h]
                            nc.scalar.activation(
                                exT[jt][:, lo - c0:w1 - c0],
                                pss[h % 2][:, lo - w0:1024],
                                mybir.ActivationFunctionType.Exp)
                            if lo == c0:  # causal mask on the diagonal block
                                nc.vector.tensor_mul(
                                    exT[jt][:, 0:P], exT[jt][:, 0:P],
                                    mk_sb[:])

            def norm_ib(h, ib, avt):
                hb, rb = h // 2, (h % 2) * 64
                rs = ns.tile([1, 512], F32, tag="rs", name="rs_t")
                nc.scalar.copy(rs[:], avt[64:65, :])
                # broadcast rowsum across 64 partitions via K=1 matmul,
                # borrowing a scores-pool PSUM slot
                bc = sc.tile([64, 512], F32, tag="scps", name="bc_t")
                nc.tensor.matmul(bc[:], lhsT=ones_sb[:], rhs=rs[:],
                                 start=True, stop=True)
                rcb = ns.tile([64, 512], F32, tag="rcb", name="rcb_t")
                nc.vector.reciprocal_approx_fast(rcb[:], bc[:])
                nc.vector.tensor_mul(
                    outT[hb][rb:rb + 64, ib * 512:(ib + 1) * 512],
                    avt[0:64, :], rcb[:])

            def phase_av(h):
                exT = ex_tiles.pop(h)
                for half in range(2):   # ib pair sweeps: (0,1) then (2,3)
                    avps = {}
                    for ib in (2 * half, 2 * half + 1):
                        avps[ib] = av.tile([P, 512], F32, tag="avps",
                                           name="avps_t")
                        for jt in range(min(ST, 4 * ib + 4)):
                            c0 = jt * P
                            p0 = ib * 512
                            a = max(c0, p0)
                            nc.tensor.matmul(
                                avps[ib][0:65, a - p0:512],
                                lhsT=vh[jt][:, h, :],
                                rhs=exT[jt][:, a - c0:p0 + 512 - c0],
                                start=(jt == 0),
                                stop=(jt == min(ST - 1, 4 * ib + 3)))
                    for ib in (2 * half, 2 * half + 1):
                        norm_ib(h, ib, avps[ib])

            phase_scores_pair(0)
            av = actx.enter_context(
                tc.tile_pool(name="avps", bufs=2, space="PSUM"))
            phase_av(0)
            phase_scores_pair(1)
            phase_av(1)
            phase_av(2)
            phase_av(3)

        # ---------------- output projection ----------------
        with ExitStack() as octx:
            op = octx.enter_context(
                tc.tile_pool(name="ops", bufs=4, space="PSUM"))
            ob = octx.enter_context(tc.tile_pool(name="ob", bufs=3))
            for st in range(ST):
                po = [op.tile([P, 512], F32, tag="ops", name="ops_t") for _ in range(2)]
                for kb2 in range(2):
                    for nchunk in range(2):
                        nc.tensor.matmul(
                            po[nchunk][:],
                            lhsT=outT[kb2][:, st * P:(st + 1) * P],
                            rhs=wo_sb[:, kb2, nchunk * 512:(nchunk + 1) * 512],
                            start=(kb2 == 0), stop=(kb2 == 1))
                o0 = ob.tile([P, 512], F32, tag="ob", name="ob_t")
                o1 = ob.tile([P, 512], F32, tag="ob", name="ob_t")
                nc.vector.tensor_copy(o0[:], po[0][:])
                nc.scalar.copy(o1[:], po[1][:])
                nc.sync.dma_start(out[st * P:(st + 1) * P, 0:512], o0[:])
                nc.sync.dma_start(out[st * P:(st + 1) * P, 512:1024], o1[:])

    nc.compile()
    return nc


_CACHE = {}


def _get_program():
    if "nc" not in _CACHE:
        _CACHE["nc"] = _build_program()
    return _CACHE["nc"]


def _make_in_maps(v, k, q, mask):
    """Host-side shard prep. Returns per-core input maps + folded bias."""
    inputs = _CACHE["inputs"]
    wq_w, wq_b = inputs["wq_w"], inputs["wq_b"]
    wk_w, wk_b = inputs["wk_w"], inputs["wk_b"]
    wv_w, wv_b = inputs["wv_w"], inputs["wv_b"]
    wo_w, wo_b = inputs["wo_w"], inputs["wo_b"]

    scale = np.float32(1.0 / np.sqrt(DEPTH))
    mk_np = np.where(np.arange(P)[:, None] > np.arange(P)[None, :],
                     0.0, 1.0).astype(NP_BF16)

    qTs = [np.ascontiguousarray(np.asarray(q[b]).T).astype(NP_BF16)
           for b in range(B)]
    kTs = [np.ascontiguousarray(np.asarray(k[b]).T).astype(NP_BF16)
           for b in range(B)]
    vTs = [np.ascontiguousarray(np.asarray(v[b]).T).astype(NP_BF16)
           for b in range(B)]

    in_maps = []
    for c in range(N_CORES):
        b, g = c // HPC, c % HPC
        c0 = g * DHC
        in_maps.append({
            "qT": qTs[b], "kT": kTs[b], "vT": vTs[b],
            "wq": np.ascontiguousarray(
                wq_w[:, c0:c0 + DHC] * scale).astype(NP_BF16),
            "wk": np.ascontiguousarray(wk_w[:, c0:c0 + DHC]).astype(NP_BF16),
            "wv": np.ascontiguousarray(wv_w[:, c0:c0 + DHC]).astype(NP_BF16),
            "wo": np.ascontiguousarray(wo_w[c0:c0 + DHC, :]).astype(NP_BF16),
            "qb": np.ascontiguousarray(wq_b[c0:c0 + DHC] * scale),
            "kb": np.ascontiguousarray(wk_b[c0:c0 + DHC]),
            "mk": mk_np,
        })
    bias_eff = (wo_b + wv_b @ wo_w).astype(np.float32)
    return in_maps, bias_eff


def run(v, k, q, mask, trace=False, tmpdir=None):
    nc = _get_program()
    in_maps, bias_eff = _make_in_maps(v, k, q, mask)
    res = run_bass_kernel_spmd(nc, in_maps, core_ids=list(range(N_CORES)),
                               trace=trace, tmpdir=tmpdir)
    outp = np.empty((B, S, D), np.float32)
    for b in range(B):
        acc = res.results[b * HPC]["out"].astype(np.float32).copy()
        for g in range(1, HPC):
            acc += res.results[b * HPC + g]["out"]
        outp[b] = acc + bias_eff[None, :]
    return outp, res


def kernel(v, k, q, mask, wq_w, wq_b, wk_w, wk_b, wv_w, wv_b, wo_w, wo_b,
           **_ignored):
    _CACHE["inputs"] = dict(wq_w=np.asarray(wq_w), wq_b=np.asarray(wq_b),
                            wk_w=np.asarray(wk_w), wk_b=np.asarray(wk_b),
                            wv_w=np.asarray(wv_w), wv_b=np.asarray(wv_b),
                            wo_w=np.asarray(wo_w), wo_b=np.asarray(wo_b))
    outp, _ = run(np.asarray(v), np.asarray(k), np.asarray(q),
                  np.asarray(mask))
    return outp


# revision 20
# speedup vs baseline: 1.0558x; 1.0558x over previous
"""Multi-head causal attention (B=2, S=2048, D=1024, H=16) on 8 TRN2 cores.

Sharding: batch x head-group. Core c handles batch c//4 and heads
4*(c%4) .. 4*(c%4)+3. Each core computes its 4 heads' attention plus the
partial output projection; the host sums the 4 partials per batch and adds
the folded bias vector.
"""

import ml_dtypes
import numpy as np
from contextlib import ExitStack

NP_BF16 = np.float16

import concourse.bass as bass
import concourse.tile as tile
from concourse import mybir, bacc
from concourse.bass_utils import run_bass_kernel_spmd

B, S, D, H = 2, 2048, 1024, 16
DEPTH = D // H            # 64
HPC = 4                   # heads per core
DHC = HPC * DEPTH         # 256 head-dims per core
N_CORES = 8
P = 128
KT = D // P               # 8 contraction tiles for projections
ST = S // P               # 16 sequence tiles
F32 = mybir.dt.float32
BF16 = mybir.dt.bfloat16
MMDT = mybir.dt.float16   # dtype for matmul operands (PSUM accum stays fp32)
NEG = -1.0e9


def _build_program():
    nc = bacc.Bacc("TRN2", target_bir_lowering=False, debug=False)

    qT = nc.dram_tensor("qT", [D, S], MMDT, kind="ExternalInput").ap()
    kT = nc.dram_tensor("kT", [D, S], MMDT, kind="ExternalInput").ap()
    vT = nc.dram_tensor("vT", [D, S], MMDT, kind="ExternalInput").ap()
    wq = nc.dram_tensor("wq", [D, DHC], MMDT, kind="ExternalInput").ap()
    wk = nc.dram_tensor("wk", [D, DHC], MMDT, kind="ExternalInput").ap()
    wv = nc.dram_tensor("wv", [D, DHC], MMDT, kind="ExternalInput").ap()
    wo = nc.dram_tensor("wo", [DHC, D], MMDT, kind="ExternalInput").ap()
    qb = nc.dram_tensor("qb", [DHC], F32, kind="ExternalInput").ap()
    kb = nc.dram_tensor("kb", [DHC], F32, kind="ExternalInput").ap()
    mk = nc.dram_tensor("mk", [P, P], MMDT, kind="ExternalInput").ap()
    out = nc.dram_tensor("out", [S, D], F32, kind="ExternalOutput").ap()

    with tile.TileContext(nc) as tc, ExitStack() as ctx:
        persist = ctx.enter_context(tc.tile_pool(name="persist", bufs=1))

        # Persistent SBUF tensors
        qhT = [persist.tile([P, S], MMDT, tag=f"qhT{i}", name=f"qhT{i}") for i in range(2)]
        khT = [persist.tile([P, S], MMDT, tag=f"khT{i}", name=f"khT{i}") for i in range(2)]
        vh = [persist.tile([P, HPC, DEPTH + 1], MMDT, tag=f"vh{st}", name=f"vh{st}")
              for st in range(ST)]
        outT = [persist.tile([P, S], MMDT, tag=f"outT{i}", name=f"outT{i}") for i in range(2)]
        wq_sb = persist.tile([P, KT, DHC], MMDT, tag="wq")
        wk_sb = persist.tile([P, KT, DHC], MMDT, tag="wk")
        wv_sb = persist.tile([P, KT, DHC], MMDT, tag="wv")
        wo_sb = persist.tile([P, 2, D], MMDT, tag="wo")
        qb_sb = persist.tile([P, 2], F32, tag="qb")
        kb_sb = persist.tile([P, 2], F32, tag="kb")
        mk_sb = persist.tile([P, P], MMDT, tag="mk")
        ones_sb = persist.tile([1, 64], F32, tag="ones")
        nc.vector.memset(ones_sb[:], 1.0)

        nc.sync.dma_start(wq_sb[:], wq.rearrange("(ko ki) n -> ki ko n", ki=P))
        nc.sync.dma_start(qb_sb[:], qb.rearrange("(ko ki) -> ki ko", ki=P))

        # ---------------- q/k projections ----------------
        pctx = ExitStack()
        inp = pctx.enter_context(tc.tile_pool(name="inp", bufs=KT))
        with pctx:
            pp2 = ExitStack()
            pp = pp2.enter_context(
                tc.tile_pool(name="projps", bufs=4, space="PSUM"))

            for src_ap, wsb, bias_sb, dst in (
                (qT, wq_sb, qb_sb, qhT),
                (kT, wk_sb, kb_sb, khT),
            ):
                if src_ap is kT:
                    nc.sync.dma_start(
                        wk_sb[:], wk.rearrange("(ko ki) n -> ki ko n", ki=P))
                    nc.sync.dma_start(
                        kb_sb[:], kb.rearrange("(ko ki) -> ki ko", ki=P))
                    nc.sync.dma_start(
                        wv_sb[:], wv.rearrange("(ko ki) n -> ki ko n", ki=P))
                    nc.sync.dma_start(
                        wo_sb[:], wo.rearrange("(ko ki) n -> ki ko n", ki=P))
                    nc.sync.dma_start(mk_sb[:], mk)
                tiles = []
                for kt in range(KT):
                    t = inp.tile([P, S], MMDT, tag="inp", name="inp_t")
                    nc.sync.dma_start(t[:], src_ap[kt * P:(kt + 1) * P, :])
                    tiles.append(t)
                for dhb in range(2):
                    ps = [pp.tile([P, 512], F32, tag="projps", name="projps_t")
                          for _ in range(4)]
                    for kt in range(KT):
                        for sb in range(4):
                            nc.tensor.matmul(
                                ps[sb][:],
                                lhsT=wsb[:, kt, dhb * P:(dhb + 1) * P],
                                rhs=tiles[kt][:, sb * 512:(sb + 1) * 512],
                                start=(kt == 0), stop=(kt == KT - 1))
                    for sb in range(4):
                        nc.vector.tensor_scalar_add(
                            dst[dhb][:, sb * 512:(sb + 1) * 512],
                            ps[sb][:], bias_sb[:, dhb:dhb + 1])
            pp2.close()

            # v input tiles (projection matmuls are zipped into the scores
            # phase below to keep the PE dense while ACT drains exps)
            vtiles = []
            for kt in range(KT):
                t = inp.tile([P, S], MMDT, tag="inp", name="inp_t")
                nc.sync.dma_start(t[:], vT[kt * P:(kt + 1) * P, :])
                vtiles.append(t)

            # ---------------- attention ----------------
            with ExitStack() as actx:
                sc = actx.enter_context(
                    tc.tile_pool(name="scps", bufs=2, space="PSUM"))
                bcp = actx.enter_context(
                    tc.tile_pool(name="bcps", bufs=1, space="PSUM"))
                ex = actx.enter_context(tc.tile_pool(name="exp", bufs=3))
                ns = actx.enter_context(tc.tile_pool(name="norm", bufs=1))

                ex_tiles = {}

                def gen_scores_pair(hb):
                    heads = (2 * hb, 2 * hb + 1)
                    for h in heads:
                        ex_tiles[h] = [ex.tile([P, S - jt * P], MMDT,
                                               tag=f"ex{jt}",
                                               name=f"ex{jt}_t")
                                       for jt in range(ST)]
                    for jt in range(ST):
                        c0 = jt * P
                        for w0 in range(0, S, 1024):
                            w1 = w0 + 1024
                            lo = max(c0, w0)
                            if lo >= w1:
                                continue
                            pss = [sc.tile([P, 1024], F32, tag="scps",
                                           name="scps_t") for _ in heads]
                            # adjacent K=64 matmuls in row groups 0-1 / 2-3
                            # run concurrently in the PE array
                            for p0 in range(w0, w1, 512):
                                a, b = max(lo, p0), p0 + 512
                                if a >= b:
                                    continue
                                for h in heads:
                                    rb = (h % 2) * 64
                                    nc.tensor.matmul(
                                        pss[h % 2][:, a - w0:b - w0],
                                        lhsT=khT[hb][rb:rb + 64, c0:c0 + P],
                                        rhs=qhT[hb][rb:rb + 64, a:b],
                                        start=True, stop=True)
                            for h in heads:
                                exT = ex_tiles[h]
                                nc.scalar.activation(
                                    exT[jt][:, lo - c0:w1 - c0],
                                    pss[h % 2][:, lo - w0:1024],
                                    mybir.ActivationFunctionType.Exp)
                                if lo == c0:  # causal mask on diagonal
                                    nc.vector.tensor_mul(
                                        exT[jt][:, 0:P], exT[jt][:, 0:P],
                                        mk_sb[:])
                            yield

                def gen_vproj():
                    for st in range(ST):
                        pv = vp.tile([P, DHC], F32, tag="vps", name="vps_t")
                        for kt in range(KT):
                            nc.tensor.matmul(
                                pv[:],
                                lhsT=vtiles[kt][:, st * P:(st + 1) * P],
                                rhs=wv_sb[:, kt, :],
                                start=(kt == 0), stop=(kt == KT - 1))
                        nc.vector.tensor_copy(
                            vh[st][:, :, 0:DEPTH],
                            pv.rearrange("p (h c) -> p h c", h=HPC))
                        nc.vector.memset(vh[st][:, :, DEPTH:DEPTH + 1], 1.0)
                        yield

                def norm_ib(h, ib, avt):
                    hb, rb = h // 2, (h % 2) * 64
                    rs = ns.tile([1, 512], F32, tag="rs", name="rs_t")
                    nc.scalar.copy(rs[:], avt[64:65, :])
                    # broadcast rowsum across 64 partitions via K=1 matmul
                    bc = bcp.tile([64, 512], F32, tag="bcps", name="bc_t")
                    nc.tensor.matmul(bc[:], lhsT=ones_sb[:], rhs=rs[:],
                                     start=True, stop=True)
                    rcb = ns.tile([64, 512], F32, tag="rcb", name="rcb_t")
                    nc.vector.reciprocal_approx_fast(rcb[:], bc[:])
                    nc.vector.tensor_mul(
                        outT[hb][rb:rb + 64, ib * 512:(ib + 1) * 512],
                        avt[0:64, :], rcb[:])

                def gen_av_pair(h0, h1):
                    for h in (h0, h1):
                        exT = ex_tiles.pop(h)
                        for half in range(2):  # ib sweeps (0,1) then (2,3)
                            avps = {}
                            for ib in (2 * half, 2 * half + 1):
                                avt = av.tile([P, 512], F32, tag="avps",
                                              name="avps_t")
                                avps[ib] = avt
                                cnt = 0
                                for jt in range(min(ST, 4 * ib + 4)):
                                    c0 = jt * P
                                    p0 = ib * 512
                                    a = max(c0, p0)
                                    nc.tensor.matmul(
                                        avt[0:65, a - p0:512],
                                        lhsT=vh[jt][:, h, :],
                                        rhs=exT[jt][:, a - c0:p0 + 512 - c0],
                                        start=(jt == 0),
                                        stop=(jt == min(ST - 1, 4 * ib + 3)))
                                    cnt += 1
                                    if cnt % 2 == 0:
                                        yield
                            for ib in (2 * half, 2 * half + 1):
                                norm_ib(h, ib, avps[ib])
                                yield

                # zip 1: scores of heads 0/1 with the v projection
                with ExitStack() as vctx:
                    vp = vctx.enter_context(
                        tc.tile_pool(name="vps", bufs=2, space="PSUM"))
                    vg = gen_vproj()
                    for i, _ in enumerate(gen_scores_pair(0)):
                        if i % 3 == 2:
                            next(vg, None)
                    for _ in vg:
                        pass

                av = actx.enter_context(
                    tc.tile_pool(name="avps", bufs=2, space="PSUM"))

                # zip 2: scores of heads 2/3 with A@V of heads 0/1
                ag = gen_av_pair(0, 1)
                for _ in gen_scores_pair(1):
                    next(ag, None)
                for _ in ag:
                    pass

                # tail: A@V of heads 2/3
                for _ in gen_av_pair(2, 3):
                    pass

        # ---------------- output projection ----------------
        with ExitStack() as octx:
            op = octx.enter_context(
                tc.tile_pool(name="ops", bufs=4, space="PSUM"))
            ob = octx.enter_context(tc.tile_pool(name="ob", bufs=3))
            for st in range(ST):
                po = [op.tile([P, 512], F32, tag="ops", name="ops_t") for _ in range(2)]
                for kb2 in range(2):
                    for nchunk in range(2):
                        nc.tensor.matmul(
                            po[nchunk][:],
                            lhsT=outT[kb2][:, st * P:(st + 1) * P],
                            rhs=wo_sb[:, kb2, nchunk * 512:(nchunk + 1) * 512],
                            start=(kb2 == 0), stop=(kb2 == 1))
                o0 = ob.tile([P, 512], F32, tag="ob", name="ob_t")
                o1 = ob.tile([P, 512], F32, tag="ob", name="ob_t")
                nc.vector.tensor_copy(o0[:], po[0][:])
                nc.scalar.copy(o1[:], po[1][:])
                nc.sync.dma_start(out[st * P:(st + 1) * P, 0:512], o0[:])
                nc.sync.dma_start(out[st * P:(st + 1) * P, 512:1024], o1[:])

    nc.compile()
    return nc


_CACHE = {}


def _get_program():
    if "nc" not in _CACHE:
        _CACHE["nc"] = _build_program()
    return _CACHE["nc"]


def _make_in_maps(v, k, q, mask):
    """Host-side shard prep. Returns per-core input maps + folded bias."""
    inputs = _CACHE["inputs"]
    wq_w, wq_b = inputs["wq_w"], inputs["wq_b"]
    wk_w, wk_b = inputs["wk_w"], inputs["wk_b"]
    wv_w, wv_b = inputs["wv_w"], inputs["wv_b"]
    wo_w, wo_b = inputs["wo_w"], inputs["wo_b"]

    scale = np.float32(1.0 / np.sqrt(DEPTH))
    mk_np = np.where(np.arange(P)[:, None] > np.arange(P)[None, :],
                     0.0, 1.0).astype(NP_BF16)

    qTs = [np.ascontiguousarray(np.asarray(q[b]).T).astype(NP_BF16)
           for b in range(B)]
    kTs = [np.ascontiguousarray(np.asarray(k[b]).T).astype(NP_BF16)
           for b in range(B)]
    vTs = [np.ascontiguousarray(np.asarray(v[b]).T).astype(NP_BF16)
           for b in range(B)]

    in_maps = []
    for c in range(N_CORES):
        b, g = c // HPC, c % HPC
        c0 = g * DHC
        in_maps.append({
            "qT": qTs[b], "kT": kTs[b], "vT": vTs[b],
            "wq": np.ascontiguousarray(
                wq_w[:, c0:c0 + DHC] * scale).astype(NP_BF16),
            "wk": np.ascontiguousarray(wk_w[:, c0:c0 + DHC]).astype(NP_BF16),
            "wv": np.ascontiguousarray(wv_w[:, c0:c0 + DHC]).astype(NP_BF16),
            "wo": np.ascontiguousarray(wo_w[c0:c0 + DHC, :]).astype(NP_BF16),
            "qb": np.ascontiguousarray(wq_b[c0:c0 + DHC] * scale),
            "kb": np.ascontiguousarray(wk_b[c0:c0 + DHC]),
            "mk": mk_np,
        })
    bias_eff = (wo_b + wv_b @ wo_w).astype(np.float32)
    return in_maps, bias_eff


def run(v, k, q, mask, trace=False, tmpdir=None):
    nc = _get_program()
    in_maps, bias_eff = _make_in_maps(v, k, q, mask)
    res = run_bass_kernel_spmd(nc, in_maps, core_ids=list(range(N_CORES)),
                               trace=trace, tmpdir=tmpdir)
    outp = np.empty((B, S, D), np.float32)
    for b in range(B):
        acc = res.results[b * HPC]["out"].astype(np.float32).copy()
        for g in range(1, HPC):
            acc += res.results[b * HPC + g]["out"]
        outp[b] = acc + bias_eff[None, :]
    return outp, res


def kernel(v, k, q, mask, wq_w, wq_b, wk_w, wk_b, wv_w, wv_b, wo_w, wo_b,
           **_ignored):
    _CACHE["inputs"] = dict(wq_w=np.asarray(wq_w), wq_b=np.asarray(wq_b),
                            wk_w=np.asarray(wk_w), wk_b=np.asarray(wk_b),
                            wv_w=np.asarray(wv_w), wv_b=np.asarray(wv_b),
                            wo_w=np.asarray(wo_w), wo_b=np.asarray(wo_b))
    outp, _ = run(np.asarray(v), np.asarray(k), np.asarray(q),
                  np.asarray(mask))
    return outp


# revision 22
# speedup vs baseline: 1.2517x; 1.1855x over previous
"""Multi-head causal attention (B=2, S=2048, D=1024, H=16) on 8 TRN2 cores.

Sharding: batch x head-group. Core c handles batch c//4 and heads
4*(c%4) .. 4*(c%4)+3. Each core computes its 4 heads' attention plus the
partial output projection; the host sums the 4 partials per batch and adds
the folded bias vector.
"""

import ml_dtypes
import numpy as np
from contextlib import ExitStack

NP_BF16 = np.float16

import concourse.bass as bass
import concourse.tile as tile
from concourse import mybir, bacc
from concourse.bass_utils import run_bass_kernel_spmd

B, S, D, H = 2, 2048, 1024, 16
DEPTH = D // H            # 64
HPC = 4                   # heads per core
DHC = HPC * DEPTH         # 256 head-dims per core
N_CORES = 8
P = 128
KT = D // P               # 8 contraction tiles for projections
ST = S // P               # 16 sequence tiles
F32 = mybir.dt.float32
BF16 = mybir.dt.bfloat16
MMDT = mybir.dt.float16   # dtype for matmul operands (PSUM accum stays fp32)
NEG = -1.0e9


def _build_program():
    nc = bacc.Bacc("TRN2", target_bir_lowering=False, debug=False)

    qT = nc.dram_tensor("qT", [D, S], MMDT, kind="ExternalInput").ap()
    kT = nc.dram_tensor("kT", [D, S], MMDT, kind="ExternalInput").ap()
    vT = nc.dram_tensor("vT", [D, S], MMDT, kind="ExternalInput").ap()
    wq = nc.dram_tensor("wq", [D, DHC], MMDT, kind="ExternalInput").ap()
    wk = nc.dram_tensor("wk", [D, DHC], MMDT, kind="ExternalInput").ap()
    wv = nc.dram_tensor("wv", [D, DHC], MMDT, kind="ExternalInput").ap()
    wo = nc.dram_tensor("wo", [DHC, D], MMDT, kind="ExternalInput").ap()
    qb = nc.dram_tensor("qb", [DHC], F32, kind="ExternalInput").ap()
    kb = nc.dram_tensor("kb", [DHC], F32, kind="ExternalInput").ap()
    mk = nc.dram_tensor("mk", [P, P], MMDT, kind="ExternalInput").ap()
    out = nc.dram_tensor("out", [S, D], F32, kind="ExternalOutput").ap()

    with tile.TileContext(nc) as tc, ExitStack() as ctx:
        persist = ctx.enter_context(tc.tile_pool(name="persist", bufs=1))

        # Persistent SBUF tensors
        qhT = [persist.tile([P, S], MMDT, tag=f"qhT{i}", name=f"qhT{i}") for i in range(2)]
        khT = [persist.tile([P, S], MMDT, tag=f"khT{i}", name=f"khT{i}") for i in range(2)]
        vh = [persist.tile([P, HPC, DEPTH + 1], MMDT, tag=f"vh{st}", name=f"vh{st}")
              for st in range(ST)]
        outT = [persist.tile([P, S], MMDT, tag=f"outT{i}", name=f"outT{i}") for i in range(2)]
        wq_sb = persist.tile([P, KT, DHC], MMDT, tag="wq")
        wk_sb = persist.tile([P, KT, DHC], MMDT, tag="wk")
        wv_sb = persist.tile([P, KT, DHC], MMDT, tag="wv")
        wo_sb = persist.tile([P, 2, D], MMDT, tag="wo")
        qb_sb = persist.tile([P, 2], F32, tag="qb")
        kb_sb = persist.tile([P, 2], F32, tag="kb")
        mk_sb = persist.tile([P, P], MMDT, tag="mk")
        ones_sb = persist.tile([1, 64], F32, tag="ones")
        nc.vector.memset(ones_sb[:], 1.0)

        nc.sync.dma_start(wq_sb[:], wq.rearrange("(ko ki) n -> ki ko n", ki=P))
        nc.sync.dma_start(qb_sb[:], qb.rearrange("(ko ki) -> ki ko", ki=P))

        # ---------------- q/k projections ----------------
        pctx = ExitStack()
        inp = pctx.enter_context(tc.tile_pool(name="inp", bufs=KT, side="right"))
        if True:
            pp2 = ExitStack()
            pp = pp2.enter_context(
                tc.tile_pool(name="projps", bufs=4, space="PSUM"))

            for src_ap, wsb, bias_sb, dst in (
                (qT, wq_sb, qb_sb, qhT),
                (kT, wk_sb, kb_sb, khT),
            ):
                if src_ap is kT:
                    nc.sync.dma_start(
                        wk_sb[:], wk.rearrange("(ko ki) n -> ki ko n", ki=P))
                    nc.sync.dma_start(
                        kb_sb[:], kb.rearrange("(ko ki) -> ki ko", ki=P))
                    nc.sync.dma_start(
                        wv_sb[:], wv.rearrange("(ko ki) n -> ki ko n", ki=P))
                    nc.sync.dma_start(
                        wo_sb[:], wo.rearrange("(ko ki) n -> ki ko n", ki=P))
                    nc.sync.dma_start(mk_sb[:], mk)
                tiles = []
                for kt in range(KT):
                    t = inp.tile([P, S], MMDT, tag="inp", name="inp_t")
                    nc.sync.dma_start(t[:], src_ap[kt * P:(kt + 1) * P, :])
                    tiles.append(t)
                for dhb in range(2):
                    ps = [pp.tile([P, 512], F32, tag="projps", name="projps_t")
                          for _ in range(4)]
                    for kt in range(KT):
                        for sb in range(4):
                            nc.tensor.matmul(
                                ps[sb][:],
                                lhsT=wsb[:, kt, dhb * P:(dhb + 1) * P],
                                rhs=tiles[kt][:, sb * 512:(sb + 1) * 512],
                                start=(kt == 0), stop=(kt == KT - 1))
                    for sb in range(4):
                        nc.vector.tensor_scalar_add(
                            dst[dhb][:, sb * 512:(sb + 1) * 512],
                            ps[sb][:], bias_sb[:, dhb:dhb + 1])
            pp2.close()

            # v input tiles (projection matmuls are zipped into the scores
            # phase below to keep the PE dense while ACT drains exps)
            vtiles = []
            for kt in range(KT):
                t = inp.tile([P, S], MMDT, tag="inp", name="inp_t")
                nc.sync.dma_start(t[:], vT[kt * P:(kt + 1) * P, :])
                vtiles.append(t)

            # ---------------- attention ----------------
            with ExitStack() as actx:
                sc = actx.enter_context(
                    tc.tile_pool(name="scps", bufs=3, space="PSUM"))
                ex = actx.enter_context(tc.tile_pool(name="exp", bufs=3))
                ns = actx.enter_context(tc.tile_pool(name="norm", bufs=2))

                ex_tiles = {}

                def alloc_ex(h):
                    ex_tiles[h] = [ex.tile([P, S - jt * P], MMDT,
                                           tag=f"ex{jt}", name=f"ex{jt}_t")
                                   for jt in range(ST)]

                def gen_scores(heads):
                    hb = heads[0] // 2
                    for h in heads:
                        alloc_ex(h)
                    for jt in range(ST):
                        c0 = jt * P
                        for w0 in range(0, S, 1024):
                            w1 = w0 + 1024
                            lo = max(c0, w0)
                            if lo >= w1:
                                continue
                            pss = {h: sc.tile([P, 1024], F32, tag="scps",
                                              name="scps_t") for h in heads}
                            # adjacent K=64 matmuls in row groups 0-1 / 2-3
                            # run concurrently in the PE array
                            for p0 in range(w0, w1, 512):
                                a, b = max(lo, p0), p0 + 512
                                if a >= b:
                                    continue
                                for h in heads:
                                    rb = (h % 2) * 64
                                    nc.tensor.matmul(
                                        pss[h][:, a - w0:b - w0],
                                        lhsT=khT[hb][rb:rb + 64, c0:c0 + P],
                                        rhs=qhT[hb][rb:rb + 64, a:b],
                                        start=True, stop=True)
                            for h in heads:
                                exT = ex_tiles[h]
                                nc.scalar.activation(
                                    exT[jt][:, lo - c0:w1 - c0],
                                    pss[h][:, lo - w0:1024],
                                    mybir.ActivationFunctionType.Exp)
                                if lo == c0:  # causal mask on diagonal
                                    nc.vector.tensor_mul(
                                        exT[jt][:, 0:P], exT[jt][:, 0:P],
                                        mk_sb[:])
                            yield

                def gen_vproj():
                    for st in range(ST):
                        pv = vp.tile([P, DHC], F32, tag="vps", name="vps_t")
                        for kt in range(KT):
                            nc.tensor.matmul(
                                pv[:],
                                lhsT=vtiles[kt][:, st * P:(st + 1) * P],
                                rhs=wv_sb[:, kt, :],
                                start=(kt == 0), stop=(kt == KT - 1))
                        nc.vector.tensor_copy(
                            vh[st][:, :, 0:DEPTH],
                            pv.rearrange("p (h c) -> p h c", h=HPC))
                        nc.vector.memset(vh[st][:, :, DEPTH:DEPTH + 1], 1.0)
                        yield

                def norm_ib(h, ib, avt):
                    hb, rb = h // 2, (h % 2) * 64
                    rs = ns.tile([1, 512], F32, tag="rs", name="rs_t")
                    nc.scalar.copy(rs[:], avt[64:65, :])
                    rc1 = ns.tile([1, 512], F32, tag="rc1", name="rc1_t")
                    nc.vector.reciprocal_approx_fast(rc1[:], rs[:])
                    rcb = ns.tile([64, 512], F32, tag="rcb", name="rcb_t")
                    nc.gpsimd.partition_broadcast(rcb[:], rc1[0:1, :],
                                                  channels=64)
                    nc.vector.tensor_mul(
                        outT[hb][rb:rb + 64, ib * 512:(ib + 1) * 512],
                        avt[0:64, :], rcb[:])

                def gen_av(h, after_half=None):
                    exT = ex_tiles.pop(h)
                    for half in range(2):  # ib sweeps (0,1) then (2,3)
                        avps = {}
                        for ib in (2 * half, 2 * half + 1):
                            avt = av.tile([P, 512], F32, tag="avps",
                                          name="avps_t")
                            avps[ib] = avt
                            cnt = 0
                            for jt in range(min(ST, 4 * ib + 4)):
                                c0 = jt * P
                                p0 = ib * 512
                                a = max(c0, p0)
                                nc.tensor.matmul(
                                    avt[0:65, a - p0:512],
                                    lhsT=vh[jt][:, h, :],
                                    rhs=exT[jt][:, a - c0:p0 + 512 - c0],
                                    start=(jt == 0),
                                    stop=(jt == min(ST - 1, 4 * ib + 3)))
                                cnt += 1
                                if cnt % 2 == 0:
                                    yield
                        for ib in (2 * half, 2 * half + 1):
                            norm_ib(h, ib, avps[ib])
                            yield
                        if after_half is not None:
                            after_half(half)

                def zip_gens(main, aux, ratio=1.0):
                    """Drive main; pull `ratio` aux steps per main step."""
                    debt = 0.0
                    for _ in main:
                        debt += ratio
                        while debt >= 1.0:
                            next(aux, None)
                            debt -= 1.0
                    for _ in aux:
                        pass

                def chain(*gens):
                    for g in gens:
                        yield from g

                # zip 1: paired scores of heads 0/1 with the v projection
                with ExitStack() as vctx:
                    vp = vctx.enter_context(
                        tc.tile_pool(name="vps", bufs=2, space="PSUM"))
                    zip_gens(gen_scores((0, 1)), gen_vproj(), ratio=0.4)

                pctx.close()   # q/k/v input tiles no longer needed

                av = actx.enter_context(
                    tc.tile_pool(name="avps", bufs=2, space="PSUM"))
                ob = actx.enter_context(tc.tile_pool(name="ob", bufs=4))

                # zip 2: scores of head 2 with A@V of heads 0 and 1
                zip_gens(gen_scores((2,)), chain(gen_av(0), gen_av(1)),
                         ratio=1.3)
                # zip 3: scores of head 3 with A@V of head 2
                zip_gens(gen_scores((3,)), gen_av(2), ratio=0.7)

                # zip 4: A@V of head 3 with the output projection
                def outproj_sts(half):
                    for st in range(8 * half, 8 * half + 8):
                        po = [sc.tile([P, 1024], F32, tag="scps",
                                      name="scps_t") for _ in range(1)]
                        pot = po[0]
                        for kb2 in range(2):
                            for nchunk in range(2):
                                nc.tensor.matmul(
                                    pot[:, nchunk * 512:(nchunk + 1) * 512],
                                    lhsT=outT[kb2][:, st * P:(st + 1) * P],
                                    rhs=wo_sb[:, kb2,
                                              nchunk * 512:(nchunk + 1) * 512],
                                    start=(kb2 == 0), stop=(kb2 == 1))
                        o = ob.tile([P, D], F32, tag="ob", name="ob_t")
                        nc.vector.tensor_copy(o[:, 0:512], pot[:, 0:512])
                        nc.scalar.copy(o[:, 512:1024], pot[:, 512:1024])
                        nc.sync.dma_start(out[st * P:(st + 1) * P, :], o[:])

                for _ in gen_av(3, after_half=outproj_sts):
                    pass

    nc.compile()
    return nc


_CACHE = {}


def _get_program():
    if "nc" not in _CACHE:
        _CACHE["nc"] = _build_program()
    return _CACHE["nc"]


def _make_in_maps(v, k, q, mask):
    """Host-side shard prep. Returns per-core input maps + folded bias."""
    inputs = _CACHE["inputs"]
    wq_w, wq_b = inputs["wq_w"], inputs["wq_b"]
    wk_w, wk_b = inputs["wk_w"], inputs["wk_b"]
    wv_w, wv_b = inputs["wv_w"], inputs["wv_b"]
    wo_w, wo_b = inputs["wo_w"], inputs["wo_b"]

    scale = np.float32(1.0 / np.sqrt(DEPTH))
    mk_np = np.where(np.arange(P)[:, None] > np.arange(P)[None, :],
                     0.0, 1.0).astype(NP_BF16)

    qTs = [np.ascontiguousarray(np.asarray(q[b]).T).astype(NP_BF16)
           for b in range(B)]
    kTs = [np.ascontiguousarray(np.asarray(k[b]).T).astype(NP_BF16)
           for b in range(B)]
    vTs = [np.ascontiguousarray(np.asarray(v[b]).T).astype(NP_BF16)
           for b in range(B)]

    in_maps = []
    for c in range(N_CORES):
        b, g = c // HPC, c % HPC
        c0 = g * DHC
        in_maps.append({
            "qT": qTs[b], "kT": kTs[b], "vT": vTs[b],
            "wq": np.ascontiguousarray(
                wq_w[:, c0:c0 + DHC] * scale).astype(NP_BF16),
            "wk": np.ascontiguousarray(wk_w[:, c0:c0 + DHC]).astype(NP_BF16),
            "wv": np.ascontiguousarray(wv_w[:, c0:c0 + DHC]).astype(NP_BF16),
            "wo": np.ascontiguousarray(wo_w[c0:c0 + DHC, :]).astype(NP_BF16),
            "qb": np.ascontiguousarray(wq_b[c0:c0 + DHC] * scale),
            "kb": np.ascontiguousarray(wk_b[c0:c0 + DHC]),
            "mk": mk_np,
        })
    bias_eff = (wo_b + wv_b @ wo_w).astype(np.float32)
    return in_maps, bias_eff


def run(v, k, q, mask, trace=False, tmpdir=None):
    nc = _get_program()
    in_maps, bias_eff = _make_in_maps(v, k, q, mask)
    res = run_bass_kernel_spmd(nc, in_maps, core_ids=list(range(N_CORES)),
                               trace=trace, tmpdir=tmpdir)
    outp = np.empty((B, S, D), np.float32)
    for b in range(B):
        acc = res.results[b * HPC]["out"].astype(np.float32).copy()
        for g in range(1, HPC):
            acc += res.results[b * HPC + g]["out"]
        outp[b] = acc + bias_eff[None, :]
    return outp, res


def kernel(v, k, q, mask, wq_w, wq_b, wk_w, wk_b, wv_w, wv_b, wo_w, wo_b,
           **_ignored):
    _CACHE["inputs"] = dict(wq_w=np.asarray(wq_w), wq_b=np.asarray(wq_b),
                            wk_w=np.asarray(wk_w), wk_b=np.asarray(wk_b),
                            wv_w=np.asarray(wv_w), wv_b=np.asarray(wv_b),
                            wo_w=np.asarray(wo_w), wo_b=np.asarray(wo_b))
    outp, _ = run(np.asarray(v), np.asarray(k), np.asarray(q),
                  np.asarray(mask))
    return outp
